# revision 1
# baseline (speedup 1.0000x reference)
"""Masked weighted-NLL loss kernel for TRN2 — v6: raw bass, no TileContext.

Same dataflow as v5 (12 indirect gathers + Ln + weighted reduce to scalar),
but hand-scheduled with explicit semaphores to cut the TileContext scaffold
visible in the v4/v5 traces (pool DRAINs, extra all-engine rendezvous,
per-sem epilogue resets) and to dispatch the input load at the earliest
possible Sync slot.
"""

import numpy as np

B, T, V = 64, 188, 32000
N_CORES = 8
B_LOC = B // N_CORES      # 8 batch rows per core
BETA = 2.0
P = 128
NK = B_LOC * T            # 1504 gathered elements per core
NCOL = (NK + P - 1) // P  # 12 columns

_NC_CACHE = None


def _build_nc():
    import concourse.bacc as bacc
    import concourse.bass as bass
    import concourse.mybir as mybir

    nc = bacc.Bacc(
        "TRN2", target_bir_lowering=False, debug=False, num_swdge_queues=2
    )

    def indirect_on_queue(out, in_, off_ap, element_offset, queue):
        """bass.indirect_dma_start with a selectable SWDGE queue name.

        Mirrors concourse.bass GpSimd.indirect_dma_start for the gather case
        (in_ [N,1], axis=0 -> coef=1) but emits on `queue` so half the
        descriptor generation can land on the second SWDGE context.
        """
        gp = nc.gpsimd
        out_l = gp.lower_ap_dma(out, for_indirect_dma=True)
        in_l = gp.lower_ap_dma(in_, for_indirect_dma=True)
        assert len(in_l) == 1 and len(out_l) == 1
        off_l = gp.lower_ap_dma(off_ap)
        assert len(off_l) == 1
        in_l.append(off_l[0])
        in_l[0].dynamic_ap_info = mybir.DynamicAccessPatternInfo(
            c=element_offset,
            actual_ap=out.ap,
            indirect_dim_max_index=in_.shape[0],
            offset_expr=[
                mybir.DynamicAccessPatternOffsetExpr(
                    coef=1,
                    aff_expr=mybir.DynamicAccessPatternOffsetExprAffExpr(
                        kind="IndirectArgId", arg_id=1
                    ),
                )
            ],
        )
        return gp.add_instruction(
            mybir.InstDMACopy(
                name=nc.get_next_instruction_name(),
                queue=queue,
                mode="Copy",
                ins=in_l,
                outs=out_l,
                oob_is_err=True,
                cce_op=mybir.AluOpType.bypass,
            )
        )

    scores = nc.dram_tensor(
        "scores", [B_LOC * T * V, 1], mybir.dt.float32, kind="ExternalInput"
    )
    pk_d = nc.dram_tensor("pk", [P, 2 * NCOL], mybir.dt.int32, kind="ExternalInput")
    out = nc.dram_tensor("out", [1, 1], mybir.dt.float32, kind="ExternalOutput")

    f32 = mybir.dt.float32
    i32 = mybir.dt.int32
    Alu = mybir.AluOpType
    Ln = mybir.ActivationFunctionType.Ln

    def full(t):
        sh = t.shape
        return bass.AP(t, 0, [[sh[1], sh[0]], [1, sh[1]]])

    def col(t, j0, n):
        sh = t.shape
        return bass.AP(t, j0, [[sh[1], sh[0]], [1, n]])

    with (
        nc.semaphore("ones_sem") as ones_sem,
        nc.semaphore("warm_sem") as warm_sem,
        nc.semaphore("load_sem") as load_sem,
        nc.semaphore("g_sem") as g_sem,
        nc.semaphore("act_sem") as act_sem,
        nc.semaphore("tt_sem") as tt_sem,
        nc.semaphore("mm_sem") as mm_sem,
        nc.semaphore("red_sem") as red_sem,
        nc.semaphore("out_sem") as out_sem,
        nc.sbuf_tensor([P, 1], f32) as ones,
        nc.sbuf_tensor([P, 1], f32) as scratch,
        nc.sbuf_tensor([P, 1], mybir.dt.int32) as woffs,
        nc.sbuf_tensor([P, 1], f32) as wg,
        nc.sbuf_tensor([P, 2 * NCOL], i32) as pk,
        nc.sbuf_tensor([P, NCOL], f32) as g,
        nc.sbuf_tensor([P, NCOL], f32) as logg,
        nc.sbuf_tensor([P, NCOL], f32) as prod,
        nc.psum_tensor([1, NCOL], f32) as colsum,
        nc.sbuf_tensor([1, 1], f32) as res,
    ):
        with nc.Block() as block:

            @block.sync
            def _(sync):
                sync.dma_start(full(pk), full(pk_d)).then_inc(load_sem, 16)

            @block.vector
            def _(vector):
                vector.memset(full(ones), 1.0).then_inc(ones_sem, 1)

            @block.scalar
            def _(scalar):
                # Ln table prefetch (dummy): compiler emits ACT_TABLE_LOAD here
                scalar.wait_ge(ones_sem, 1)
                scalar.activation(full(scratch), full(ones), Ln)

            @block.gpsimd
            def _(gpsimd):
                # warm-up: dummy gather from scores[0]*128 during the idle
                # window while the pk load is in flight — absorbs the first
                # op's Q7/SWDGE warm-up cost off the critical path
                gpsimd.memset(full(woffs), 0).then_inc(warm_sem, 1)
                gpsimd.wait_ge(warm_sem, 1)
                indirect_on_queue(full(wg), full(scores), full(woffs), 0,
                                  "qPoolDynamic").then_inc(warm_sem, 16)
                gpsimd.wait_ge(load_sem, 16)
                for j in range(NCOL):
                    q = "qPoolDynamic" if j % 2 == 0 else "qPoolDynamic1"
                    indirect_on_queue(
                        col(g, j, 1),
                        full(scores),
                        col(pk, j, 1),
                        j * P * V,
                        q,
                    ).then_inc(g_sem, 16)

            @block.scalar
            def _(scalar):
                scalar.wait_ge(g_sem, 16 * NCOL)
                scalar.activation(full(logg), full(g), Ln).then_inc(act_sem, 1)

            @block.vector
            def _(vector):
                vector.wait_ge(act_sem, 1)
                vector.tensor_tensor(
                    out=full(prod),
                    in0=full(logg),
                    in1=col(pk, NCOL, NCOL).bitcast(f32),
                    op=Alu.mult,
                ).then_inc(tt_sem, 1)

            @block.tensor
            def _(tensor):
                tensor.wait_ge(tt_sem, 1)
                tensor.matmul(
                    full(colsum), full(ones), full(prod), start=True, stop=True
                ).then_inc(mm_sem, 1)

            @block.vector
            def _(vector):
                vector.wait_ge(mm_sem, 1)
                vector.reduce_sum(
                    out=full(res), in_=full(colsum), axis=mybir.AxisListType.X
                ).then_inc(red_sem, 1)

            @block.sync
            def _(sync):
                # no explicit completion wait on the store: the framework
                # epilogue's dma_reset/DRAIN guarantees the write lands
                # before the NEFF completes, and skipping the wait lets the
                # semaphore-sweep epilogue start ~1us earlier
                sync.wait_ge(red_sem, 1)
                sync.dma_start(full(out), full(res)).then_inc(out_sem, 16)

    nc.compile()
    return nc


def _shard_inputs(targets_scores, targets_ground_truth, lengths):
    s = np.ascontiguousarray(targets_scores, dtype=np.float32).reshape(
        N_CORES, B_LOC * T * V, 1
    )
    gt = np.ascontiguousarray(targets_ground_truth).astype(np.int64).reshape(
        N_CORES, NK
    )
    ln = np.ascontiguousarray(lengths).astype(np.int64).reshape(N_CORES, B_LOC)

    # slot (p,j) covers k = 128*j + p; pad slots clamp to k=1503 (mw=0 there)
    kk = np.arange(P)[:, None] + P * np.arange(NCOL)[None, :]  # [P, NCOL]
    kc = np.minimum(kk, NK - 1)
    b = kc // T
    t = kc % T
    in_maps = []
    for c in range(N_CORES):
        gtc = gt[c][kc]                                          # [P, NCOL]
        offs = ((kc - P * np.arange(NCOL)[None, :]) * V + gtc).astype(np.int32)
        mask = (t < ln[c][b]) & (kk < NK)
        w = np.where(gtc == 0, 1.0, BETA)
        mw = (-(mask * w) / B).astype(np.float32)
        pk = np.concatenate([offs, mw.view(np.int32)], axis=1)   # [P, 2*NCOL]
        in_maps.append({"scores": s[c], "pk": np.ascontiguousarray(pk)})
    return in_maps


def _partial_f64(in_map):
    """Host reference for one core's partial sum (used by sim_bench)."""
    s = in_map["scores"].reshape(-1).astype(np.float64)
    offs = in_map["pk"][:, :NCOL].astype(np.int64) + (P * V) * np.arange(NCOL)[None, :]
    mw = in_map["pk"][:, NCOL:].view(np.float32).astype(np.float64)
    return np.sum(mw * np.log(s[offs]))


def _run(targets_scores, targets_ground_truth, lengths, trace=False, **spmd_kwargs):
    from concourse.bass_utils import run_bass_kernel_spmd

    global _NC_CACHE
    if _NC_CACHE is None:
        _NC_CACHE = _build_nc()
    in_maps = _shard_inputs(targets_scores, targets_ground_truth, lengths)
    return run_bass_kernel_spmd(
        _NC_CACHE,
        in_maps,
        core_ids=list(range(N_CORES)),
        trace=trace,
        **spmd_kwargs,
    )


def kernel(targets_scores, targets_ground_truth, lengths):
    r = _run(targets_scores, targets_ground_truth, lengths)
    total = np.sum(
        [np.sum(res["out"], dtype=np.float64) for res in r.results], dtype=np.float64
    )
    return np.array([total], dtype=np.float32)

